# revision 11
# baseline (speedup 1.0000x reference)
"""Trainium2 Bass kernel for nn_MultiHeadFactorizedRandomAttention.

Math: the reference builds scores = diag(sum_r l*r) (an [N,N] diagonal
matrix per (b,h)) and softmaxes it. A diagonal-score softmax has the
closed form

    out_i = ((e^{d_i} - 1) * v_i + sum_j v_j) / (e^{d_i} + N - 1)

so the O(N^2) attention collapses to two dense projections (x @ Wv.T,
out @ Wo.T) plus per-(head, position) scaling and a per-head column sum
of v.  Sharding: 8 cores = 4 batches x 2 sequence halves; every core
computes y[b, n_half, :] independently (no collectives).

Per-core device program (all matmuls in float32r, 1 cycle/row):
  valueT[c, n]   = sum_f WvT[f, c] * xT[f, n]          (c-block j, k-loop over f)
  S[c]           = sum_f WvT[f, c] * xs[f]             (xs = colsum of xT, on-chip)
  d[n, h]        = sum_r fl*fr ; e = exp(d)
  a = (e-1)/(e+N-1), b = 1/(e+N-1)   -> PE-transposed to [h, n]
  A_rep[c, n]    = E_j.T @ a_hn  (selector matmul replicates head rows)
  outT[c, n]     = valueT * A_rep + B_rep * S[c]
  y[n, c']       = sum_c outT[c, n] * WoT[c, c']
"""

import numpy as np
from contextlib import ExitStack

import concourse.bass as bass
import concourse.mybir as mybir
from concourse import bacc, tile
from concourse.bass_utils import run_bass_kernel_spmd

DT = mybir.dt.float32
F32R = mybir.dt.float32r
AL = bass.mybir.AluOpType
AF = mybir.ActivationFunctionType
AX = mybir.AxisListType

B, H, N, R, D = 4, 16, 1024, 64, 1024
HD = D // H          # 64
NL = N // 2          # 512 rows per core
KB = 8               # f (contraction) blocks of 128
CB = 8               # c blocks of 128
NT = NL // 128       # 4 n-tiles of 128


def build_nc():
    nc = bacc.Bacc("TRN2", target_bir_lowering=False, debug=False)

    xt = nc.dram_tensor("xt", [D, N], F32R, kind="ExternalInput")        # x[b].T, local n first
    wvtb = nc.dram_tensor("wvtb", [CB, 128, KB, 128], F32R, kind="ExternalInput")  # [j, f0, k, c0]
    wot = nc.dram_tensor("wot", [D, D], F32R, kind="ExternalInput")      # Wo.T  [c, c']
    fl = nc.dram_tensor("fl", [NL, H, R], DT, kind="ExternalInput")      # [n, h, r]
    fr = nc.dram_tensor("fr", [NL, H, R], DT, kind="ExternalInput")
    esel = nc.dram_tensor("esel", [H, CB, 128], F32R, kind="ExternalInput")
    ident = nc.dram_tensor("ident", [128, 128], DT, kind="ExternalInput")
    y = nc.dram_tensor("y", [NL, D], DT, kind="ExternalOutput")

    with tile.TileContext(nc) as tc, ExitStack() as ctx:
        const = ctx.enter_context(tc.tile_pool(name="const", bufs=1))
        xt_pool = ctx.enter_context(tc.tile_pool(name="xt", bufs=1))
        wvt_pool = ctx.enter_context(tc.tile_pool(name="wvt", bufs=1))
        wot_pool = ctx.enter_context(tc.tile_pool(name="wot", bufs=1))
        fct_pool = ctx.enter_context(tc.tile_pool(name="fct", bufs=2))
        small = ctx.enter_context(tc.tile_pool(name="small", bufs=2))
        vsb_pool = ctx.enter_context(tc.tile_pool(name="vsb", bufs=2))
        tmp_pool = ctx.enter_context(tc.tile_pool(name="tmp", bufs=2))
        out_pool = ctx.enter_context(tc.tile_pool(name="outT", bufs=CB))
        ysb_pool = ctx.enter_context(tc.tile_pool(name="ysb", bufs=2))

        # ---- constants / inputs ----
        id_sb = const.tile([128, 128], DT, tag="ident")
        nc.sync.dma_start(id_sb[:], ident[:])
        esel_sb = const.tile([H, CB, 128], F32R, tag="esel")
        nc.sync.dma_start(esel_sb[:], esel[:])

        xt_sb = []
        for k in range(KB):
            t = xt_pool.tile([128, N], F32R, tag=f"xt{k}")
            nc.sync.dma_start(t[:], xt[k * 128:(k + 1) * 128, :])
            xt_sb.append(t)

        wvt_sb = [None] * CB
        def load_wvt(j):
            t = wvt_pool.tile([128, KB, 128], F32R, tag=f"wvt{j}")
            nc.sync.dma_start(t[:], wvtb[j, :, :, :])
            wvt_sb[j] = t

        load_wvt(0)
        load_wvt(1)
        load_wvt(2)

        fl_sb, fr_sb = [], []
        for t in range(NT):
            a = fct_pool.tile([128, H, R], DT, tag="fl")
            nc.sync.dma_start(a[:], fl[t * 128:(t + 1) * 128, :, :])
            fl_sb.append(a)
            b_ = fct_pool.tile([128, H, R], DT, tag="fr")
            nc.sync.dma_start(b_[:], fr[t * 128:(t + 1) * 128, :, :])
            fr_sb.append(b_)

        for j in range(3, CB):
            load_wvt(j)

        wot_sb = []
        for j in range(CB):
            t = wot_pool.tile([128, D], F32R, tag=f"wot{j}")
            nc.sync.dma_start(t[:], wot[j * 128:(j + 1) * 128, :])
            wot_sb.append(t)

        # ---- xs = column sums of x (over all N), in f-partition layout ----
        # (padded to 2 columns per k: fp32r matmul needs an even moving free dim)
        xs = const.tile([128, KB, 2], F32R, tag="xs")
        nc.gpsimd.memset(xs[:].bitcast(DT), 0.0)
        with nc.allow_low_precision(reason="f32r is 4-byte; reduce accumulates fp32"):
            for k in range(KB):
                nc.vector.reduce_sum(xs[:, k, 0:1], xt_sb[k][:], axis=AX.X)

        # ---- factor math: d = sum_r fl*fr ; a/b coefficients ----
        a_hn = const.tile([H, NL], F32R, tag="a_hn")
        b_hn = const.tile([H, NL], F32R, tag="b_hn")
        ab_small = []   # (a_t, b_t) in [n, h] layout per n-tile
        for t in range(NT):
            prod = fct_pool.tile([128, H, R], DT, tag="prod")
            nc.vector.tensor_mul(prod[:], fl_sb[t][:], fr_sb[t][:])
            d_t = small.tile([128, H], DT, tag="d")
            nc.vector.reduce_sum(d_t[:], prod[:], axis=AX.X)
            e_t = small.tile([128, H], DT, tag="e")
            nc.scalar.activation(e_t[:], d_t[:], AF.Exp)
            den = small.tile([128, H], DT, tag="den")
            nc.vector.tensor_scalar(den[:], e_t[:], float(N - 1), None, AL.add)
            b_t = small.tile([128, H], DT, tag="bt")
            nc.vector.reciprocal(b_t[:], den[:])
            # a = (e-1)/(e+N-1) = 1 - N*b  (single fused op)
            a_t = small.tile([128, H], DT, tag="at")
            nc.vector.tensor_scalar(a_t[:], b_t[:], float(-N), 1.0, AL.mult, AL.add)
            ab_small.append((a_t, b_t))

        # ---- MM1 + per-c-block combine ----
        mm1_ctx = ExitStack()
        ps_v = mm1_ctx.enter_context(tc.tile_pool(name="ps_v", bufs=2, space="PSUM"))
        ps_s = mm1_ctx.enter_context(tc.tile_pool(name="ps_s", bufs=2, space="PSUM"))
        ps_rep = mm1_ctx.enter_context(tc.tile_pool(name="ps_rep", bufs=1, space="PSUM"))
        ps_tr = mm1_ctx.enter_context(tc.tile_pool(name="ps_tr", bufs=1, space="PSUM"))

        outT = []

        def kloop(j):
            pv = ps_v.tile([128, NL], DT, tag="pv")
            ps = ps_s.tile([128, 2], DT, tag="ps")
            for k in range(KB):
                lhs = wvt_sb[j][:, k, :]
                nc.tensor.matmul(pv[:], lhs, xt_sb[k][:, 0:NL],
                                 start=(k == 0), stop=(k == KB - 1))
                nc.tensor.matmul(ps[:], lhs, xs[:, k, :],
                                 start=(k == 0), stop=(k == KB - 1))
            return pv, ps

        def transposes():
            for t in range(NT):
                a_t, b_t = ab_small[t]
                for src, dst in ((a_t, a_hn), (b_t, b_hn)):
                    tp = ps_tr.tile([H, 128], DT, tag="tr")
                    nc.tensor.transpose(tp[:], src[:], id_sb[:])
                    nc.vector.tensor_copy(dst[:, t * 128:(t + 1) * 128], tp[:])

        def combine(j, pv, ps):
            v_sb = vsb_pool.tile([128, NL], DT, tag="vsb")
            nc.vector.tensor_copy(v_sb[:], pv[:])
            s_sb = small.tile([128, 1], DT, tag="ssb")
            nc.scalar.copy(s_sb[:], ps[:, 0:1])
            arep = ps_rep.tile([128, NL], DT, tag="arep")
            nc.tensor.matmul(arep[:], esel_sb[:, j, :], a_hn[:], start=True, stop=True)
            brep = ps_rep.tile([128, NL], DT, tag="brep")
            nc.tensor.matmul(brep[:], esel_sb[:, j, :], b_hn[:], start=True, stop=True)
            t1 = tmp_pool.tile([128, NL], DT, tag="t1")
            nc.vector.tensor_mul(t1[:], v_sb[:], arep[:])
            t2 = tmp_pool.tile([128, NL], DT, tag="t2")
            nc.vector.tensor_scalar(t2[:], brep[:], s_sb[:], None, AL.mult)
            o = out_pool.tile([128, NL], F32R, tag="outT")
            nc.vector.tensor_add(o[:], t1[:], t2[:])
            outT.append(o)

        pend = {}
        pend[0] = kloop(0)
        pend[1] = kloop(1)
        transposes()
        combine(0, *pend.pop(0))
        combine(1, *pend.pop(1))
        for j in range(2, CB):
            pv, ps = kloop(j)
            combine(j, pv, ps)

        # ---- MM2: y[n, c'] accumulated over c blocks (j-outer) ----
        mm1_ctx.close()
        with tc.tile_pool(name="ps_y", bufs=2 * NT, space="PSUM") as ps_y:
            y_ps = [ps_y.tile([128, 512], DT, tag="ypsum", name=f"y_ps{q}")
                    for q in range(2 * NT)]
            for j in range(CB):
                for i in range(NT):
                    lhs = outT[j][:, i * 128:(i + 1) * 128]
                    for h in range(2):
                        nc.tensor.matmul(y_ps[i * 2 + h][:], lhs,
                                         wot_sb[j][:, h * 512:(h + 1) * 512],
                                         start=(j == 0), stop=(j == CB - 1))
            for i in range(NT):
                y_sb = ysb_pool.tile([128, D], DT, tag="ysb")
                nc.vector.tensor_copy(y_sb[:, 0:512], y_ps[i * 2][:])
                nc.vector.tensor_copy(y_sb[:, 512:1024], y_ps[i * 2 + 1][:])
                nc.sync.dma_start(y[i * 128:(i + 1) * 128, :], y_sb[:])

    nc.compile()
    return nc


_NC_CACHE = None


def get_nc():
    global _NC_CACHE
    if _NC_CACHE is None:
        _NC_CACHE = build_nc()
    return _NC_CACHE


def make_in_maps(x, factor_l, factor_r, Wv, Wo):
    x = np.asarray(x, dtype=np.float32)
    factor_l = np.asarray(factor_l, dtype=np.float32)
    factor_r = np.asarray(factor_r, dtype=np.float32)
    Wv = np.asarray(Wv, dtype=np.float32)
    Wo = np.asarray(Wo, dtype=np.float32)

    wvt = Wv.T  # [f, c]
    # wvtb[j, f0, k, c0] = WvT[k*128+f0, j*128+c0]
    wvtb = np.ascontiguousarray(
        wvt.reshape(KB, 128, CB, 128).transpose(2, 1, 0, 3))
    wot = np.ascontiguousarray(Wo.T)

    esel = np.zeros((H, CB, 128), dtype=np.float32)
    for j in range(CB):
        for c0 in range(128):
            esel[2 * j + c0 // HD, j, c0] = 1.0
    ident = np.eye(128, dtype=np.float32)

    in_maps = []
    for core in range(8):
        b, jh = divmod(core, 2)
        sl = slice(jh * NL, (jh + 1) * NL)
        ot = slice((1 - jh) * NL, (1 - jh) * NL + NL)
        xT = x[b].T  # [f, n]
        xt_c = np.ascontiguousarray(np.concatenate([xT[:, sl], xT[:, ot]], axis=1))
        fl_c = np.ascontiguousarray(factor_l[b, :, sl, :].transpose(1, 0, 2))
        fr_c = np.ascontiguousarray(factor_r[b, :, sl, :].transpose(1, 0, 2))
        in_maps.append({
            "xt": xt_c, "wvtb": wvtb, "wot": wot,
            "fl": fl_c, "fr": fr_c, "esel": esel, "ident": ident,
        })
    return in_maps


def assemble(results):
    y = np.empty((B, N, D), dtype=np.float32)
    for core in range(8):
        b, jh = divmod(core, 2)
        y[b, jh * NL:(jh + 1) * NL, :] = results[core]["y"]
    return y


def kernel(x, factor_l, factor_r, Wv, Wo, _trace=False, **trace_kw):
    nc = get_nc()
    in_maps = make_in_maps(x, factor_l, factor_r, Wv, Wo)
    res = run_bass_kernel_spmd(nc, in_maps, core_ids=list(range(8)),
                               trace=_trace, **trace_kw)
    out = assemble(res.results)
    if _trace:
        return out, res
    return out


if __name__ == "__main__":
    # quick CoreSim check of core 0 and core 5
    from concourse.bass_interp import CoreSim
    import reference as REF

    inputs = {k: np.asarray(v) for k, v in REF.setup_inputs().items()}
    nc = get_nc()
    in_maps = make_in_maps(**inputs)

    # numpy reference (closed form validated against jax reference separately)
    x, fl, fr, Wv, Wo = (inputs["x"], inputs["factor_l"], inputs["factor_r"],
                         inputs["Wv"], inputs["Wo"])
    val = x @ Wv.T
    d = (fl * fr).sum(-1)
    e = np.exp(d)
    Z = e + (N - 1)
    S = val.reshape(B, N, H, HD).sum(1)
    a = (e - 1) / Z
    bb = 1 / Z
    v = val.reshape(B, N, H, HD).transpose(0, 2, 1, 3)
    out = a[..., None] * v + bb[..., None] * S[:, :, None, :]
    out = out.transpose(0, 2, 1, 3).reshape(B, N, D)
    want_full = out @ Wo.T

    for core in [0, 5]:
        sim = CoreSim(nc)
        for k2, v2 in in_maps[core].items():
            sim.tensor(k2)[:] = v2
        sim.simulate()
        got = np.array(sim.tensor("y"))
        b, jh = divmod(core, 2)
        want = want_full[b, jh * NL:(jh + 1) * NL, :]
        err = np.abs(got - want).max() / np.abs(want).max()
        print(f"core {core}: sim rel err {err:.3e}")


# revision 19
# speedup vs baseline: 235.3587x; 235.3587x over previous
"""Trainium2 Bass kernel for nn_MultiHeadFactorizedRandomAttention.

Math: the reference builds scores = diag(sum_r l*r) (an [N,N] diagonal
matrix per (b,h)) and softmaxes it. A diagonal-score softmax has the
closed form

    out_i = ((e^{d_i} - 1) * v_i + sum_j v_j) / (e^{d_i} + N - 1)

so the O(N^2) attention collapses to two dense projections (x @ Wv.T,
out @ Wo.T) plus per-(head, position) scaling and a per-head column sum
of v.  Sharding: 8 cores = 4 batches x 2 sequence halves; every core
computes y[b, n_half, :] independently (no collectives).

Per-core device program (all matmuls in float32r, 1 cycle/row):
  valueT[c, n]   = sum_f WvT[f, c] * xT[f, n]          (c-block j, k-loop over f)
  S[c]           = sum_f WvT[f, c] * xs[f]             (xs = colsum of xT, on-chip)
  d[n, h]        = sum_r fl*fr ; e = exp(d)
  a = (e-1)/(e+N-1), b = 1/(e+N-1)   -> PE-transposed to [h, n]
  A_rep[c, n]    = E_j.T @ a_hn  (selector matmul replicates head rows)
  outT[c, n]     = valueT * A_rep + B_rep * S[c]
  y[n, c']       = sum_c outT[c, n] * WoT[c, c']
"""

import numpy as np
from ml_dtypes import bfloat16 as _bf16
from contextlib import ExitStack

import concourse.bass as bass
import concourse.mybir as mybir
from concourse import bacc, tile
from concourse.bass_utils import run_bass_kernel_spmd

DT = mybir.dt.float32
BF16 = mybir.dt.bfloat16
F32R = mybir.dt.float32r
AL = bass.mybir.AluOpType
AF = mybir.ActivationFunctionType
AX = mybir.AxisListType

B, H, N, R, D = 4, 16, 1024, 64, 1024
HD = D // H          # 64
NL = N // 2          # 512 rows per core
KB = 8               # f (contraction) blocks of 128
CB = 8               # c blocks of 128
NT = NL // 128       # 4 n-tiles of 128


def build_nc():
    nc = bacc.Bacc("TRN2", target_bir_lowering=False, debug=False)

    xt = nc.dram_tensor("xt", [D, N], F32R, kind="ExternalInput")        # x[b].T, local n first
    wvtb = nc.dram_tensor("wvtb", [CB, 128, KB, 128], F32R, kind="ExternalInput")  # [j, f0, k, c0]
    wot = nc.dram_tensor("wot", [D, D], F32R, kind="ExternalInput")      # Wo.T  [c, c']
    # factors ship as bf16: they only produce the scores d = sum_r l*r
    # (attention weights); their error contribution to y is ~1e-6 relative.
    fl = nc.dram_tensor("fl", [NL, H, R], BF16, kind="ExternalInput")    # [n, h, r]
    fr = nc.dram_tensor("fr", [NL, H, R], BF16, kind="ExternalInput")
    esel = nc.dram_tensor("esel", [H, CB, 128], F32R, kind="ExternalInput")
    ident = nc.dram_tensor("ident", [128, 128], DT, kind="ExternalInput")
    y = nc.dram_tensor("y", [NL, D], DT, kind="ExternalOutput")

    with tile.TileContext(nc) as tc, ExitStack() as ctx:
        const = ctx.enter_context(tc.tile_pool(name="const", bufs=1))
        xt_pool = ctx.enter_context(tc.tile_pool(name="xt", bufs=1))
        wvt_pool = ctx.enter_context(tc.tile_pool(name="wvt", bufs=1))
        wot_pool = ctx.enter_context(tc.tile_pool(name="wot", bufs=1))
        fct_pool = ctx.enter_context(tc.tile_pool(name="fct", bufs=2))
        small = ctx.enter_context(tc.tile_pool(name="small", bufs=2))
        tmp_pool = ctx.enter_context(tc.tile_pool(name="tmp", bufs=2))
        out_pool = ctx.enter_context(tc.tile_pool(name="outT", bufs=CB))
        ysb_pool = ctx.enter_context(tc.tile_pool(name="ysb", bufs=2))

        # ---- constants / inputs ----
        id_sb = const.tile([128, 128], DT, tag="ident")
        nc.sync.dma_start(id_sb[:], ident[:])
        esel_sb = const.tile([H, CB, 128], F32R, tag="esel")
        nc.sync.dma_start(esel_sb[:], esel[:])

        xt_sb = []
        for k in range(KB):
            t = xt_pool.tile([128, N], F32R, tag=f"xt{k}")
            nc.sync.dma_start(t[:], xt[k * 128:(k + 1) * 128, :])
            xt_sb.append(t)

        wvt_sb = [None] * CB
        def load_wvt(j):
            t = wvt_pool.tile([128, KB, 128], F32R, tag=f"wvt{j}")
            nc.sync.dma_start(t[:], wvtb[j, :, :, :])
            wvt_sb[j] = t

        wot_sb = [None] * CB
        def load_wot(j):
            t = wot_pool.tile([128, D], F32R, tag=f"wot{j}")
            nc.sync.dma_start(t[:], wot[j * 128:(j + 1) * 128, :])
            wot_sb[j] = t

        fl_sb, fr_sb = [], []
        def load_fct(t):
            a = fct_pool.tile([128, H, R], BF16, tag="fl", bufs=NT, name=f"fl{t}")
            nc.sync.dma_start(a[:], fl[t * 128:(t + 1) * 128, :, :])
            fl_sb.append(a)
            b_ = fct_pool.tile([128, H, R], BF16, tag="fr", bufs=NT, name=f"fr{t}")
            nc.sync.dma_start(b_[:], fr[t * 128:(t + 1) * 128, :, :])
            fr_sb.append(b_)

        # xt first (MM1 j0 needs all of it); wvt0/1 so the j-loop can start;
        # factors next (transpose chain); then wvt_j pacing MM1 interleaved
        # with wot_j feeding the inline MM2 rounds; wot6/7 land last (their
        # consumer tail is shortest: final rounds + output copies).
        load_wvt(0)
        load_wvt(1)
        for t in range(NT):
            load_fct(t)
        for j in range(2, CB):
            load_wvt(j)
            load_wot(j - 2)
        load_wot(6)
        load_wot(7)

        # ---- xs = column sums of x (over all N), in f-partition layout ----
        # (padded to 2 columns per k: fp32r matmul needs an even moving free dim)
        xs = const.tile([128, KB, 2], F32R, tag="xs")
        nc.gpsimd.memset(xs[:].bitcast(DT), 0.0)
        xs_dump = fct_pool.tile([128, N], DT, tag="xsdump", bufs=1)
        with nc.allow_low_precision(reason="f32r is 4-byte; accum is fp32"):
            for k in range(KB):
                nc.scalar.activation(xs_dump[:], xt_sb[k][:], AF.Copy,
                                     accum_out=xs[:, k, 0:1])

        # ---- factor math: d = sum_r fl*fr ; a/b coefficients ----
        a_hn = const.tile([H, NL], F32R, tag="a_hn")
        b_hn = const.tile([H, NL], F32R, tag="b_hn")
        ab_small = []   # (a_t, b_t) in [n, h] layout per n-tile
        for t in range(NT):
            prod = fct_pool.tile([128, H, R], DT, tag="prod")
            nc.vector.tensor_mul(prod[:], fl_sb[t][:], fr_sb[t][:])
            d_t = small.tile([128, H], DT, tag="d")
            nc.vector.reduce_sum(d_t[:], prod[:], axis=AX.X)
            e_t = small.tile([128, H], DT, tag="e")
            nc.scalar.activation(e_t[:], d_t[:], AF.Exp)
            den = small.tile([128, H], DT, tag="den")
            nc.vector.tensor_scalar(den[:], e_t[:], float(N - 1), None, AL.add)
            b_t = small.tile([128, H], DT, tag="bt")
            nc.vector.reciprocal(b_t[:], den[:])
            # a = (e-1)/(e+N-1) = 1 - N*b  (single fused op)
            a_t = small.tile([128, H], DT, tag="at")
            nc.vector.tensor_scalar(a_t[:], b_t[:], float(-N), 1.0, AL.mult, AL.add)
            ab_small.append((a_t, b_t))

        # ---- MM1 + combine + MM2, software-pipelined over c-blocks ----
        # PSUM (8 banks): pv 1 + S 1 + rep 2 + 4 inline y banks (i=0,1).
        # y rounds lag one c-block behind MM1 so the PE never waits on the
        # DVE combine.  i=2,3 accumulate in a deferred pass reusing slots.
        ps_v = ctx.enter_context(tc.tile_pool(name="ps_v", bufs=1, space="PSUM"))
        ps_s = ctx.enter_context(tc.tile_pool(name="ps_s", bufs=1, space="PSUM"))
        ps_rep = ctx.enter_context(tc.tile_pool(name="ps_rep", bufs=1, space="PSUM"))
        ps_y = ctx.enter_context(tc.tile_pool(name="ps_y", bufs=4, space="PSUM"))

        N_INLINE = 2
        inline_i = list(range(N_INLINE))
        defer_i = list(range(N_INLINE, NT))
        outT = []
        y_ps = {}

        def kloop(j):
            pv = ps_v.tile([128, NL], DT, tag="pv")
            ps = ps_s.tile([128, 2], DT, tag="ps")
            for k in range(KB):
                lhs = wvt_sb[j][:, k, :]
                nc.tensor.matmul(pv[:], lhs, xt_sb[k][:, 0:NL],
                                 start=(k == 0), stop=(k == KB - 1))
                nc.tensor.matmul(ps[:], lhs, xs[:, k, :],
                                 start=(k == 0), stop=(k == KB - 1))
            return pv, ps

        def transposes():
            for t in range(NT):
                a_t, b_t = ab_small[t]
                for src_, dst in ((a_t, a_hn), (b_t, b_hn)):
                    tp = ps_y.tile([H, 128], DT, tag="ypsum", name="tp")
                    nc.tensor.transpose(tp[:], src_[:], id_sb[:])
                    nc.scalar.copy(dst[:, t * 128:(t + 1) * 128], tp[:])

        def rep_mms(j):
            arep = ps_rep.tile([128, NL], DT, tag="arep")
            nc.tensor.matmul(arep[:], esel_sb[:, j, :], a_hn[:], start=True, stop=True)
            brep = ps_rep.tile([128, NL], DT, tag="brep")
            nc.tensor.matmul(brep[:], esel_sb[:, j, :], b_hn[:], start=True, stop=True)
            return arep, brep

        def combine(j, pv, ps, arep, brep):
            s_sb = small.tile([128, 1], DT, tag="ssb")
            nc.scalar.copy(s_sb[:], ps[:, 0:1])
            v_sb = tmp_pool.tile([128, NL], DT, tag="vsb")
            nc.vector.tensor_copy(v_sb[:], pv[:])
            t1 = tmp_pool.tile([128, NL], DT, tag="t1")
            nc.vector.tensor_mul(t1[:], v_sb[:], arep[:])
            o = out_pool.tile([128, NL], F32R, tag="outT")
            nc.vector.scalar_tensor_tensor(o[:], brep[:], s_sb[:], t1[:],
                                           AL.mult, AL.add)
            outT.append(o)

        def y_round(j, i_list):
            for i in i_list:
                lhs = outT[j][:, i * 128:(i + 1) * 128]
                for h in range(2):
                    if j == 0:
                        y_ps[i * 2 + h] = ps_y.tile([128, 512], DT, tag="ypsum",
                                                    name=f"y_ps{i}_{h}")
                    nc.tensor.matmul(y_ps[i * 2 + h][:], lhs,
                                     wot_sb[j][:, h * 512:(h + 1) * 512],
                                     start=(j == 0), stop=(j == CB - 1))

        def y_out(i):
            y_sb = ysb_pool.tile([128, D], DT, tag="ysb")
            nc.vector.tensor_copy(y_sb[:, 0:512], y_ps[i * 2][:])
            nc.vector.tensor_copy(y_sb[:, 512:1024], y_ps[i * 2 + 1][:])
            nc.sync.dma_start(y[i * 128:(i + 1) * 128, :], y_sb[:])

        pend = {}
        pend[0] = kloop(0)
        transposes()
        pend[0] += rep_mms(0)
        combine(0, *pend.pop(0))
        for j in range(1, CB):
            pv, ps = kloop(j)
            arep, brep = rep_mms(j)
            y_round(j - 1, inline_i)     # previous block's inline MM2 round
            combine(j, pv, ps, arep, brep)
        y_round(CB - 1, inline_i)
        for i in inline_i:
            y_out(i)
        # phase B: deferred i-tiles (all operands SBUF-resident)
        for j in range(CB):
            y_round(j, defer_i)
        for i in defer_i:
            y_out(i)

    nc.compile()
    return nc


_NC_CACHE = None


def get_nc():
    global _NC_CACHE
    if _NC_CACHE is None:
        _NC_CACHE = build_nc()
    return _NC_CACHE


def make_in_maps(x, factor_l, factor_r, Wv, Wo):
    x = np.asarray(x, dtype=np.float32)
    factor_l = np.asarray(factor_l, dtype=np.float32)
    factor_r = np.asarray(factor_r, dtype=np.float32)
    Wv = np.asarray(Wv, dtype=np.float32)
    Wo = np.asarray(Wo, dtype=np.float32)

    wvt = Wv.T  # [f, c]
    # wvtb[j, f0, k, c0] = WvT[k*128+f0, j*128+c0]
    wvtb = np.ascontiguousarray(
        wvt.reshape(KB, 128, CB, 128).transpose(2, 1, 0, 3))
    wot = np.ascontiguousarray(Wo.T)

    esel = np.zeros((H, CB, 128), dtype=np.float32)
    for j in range(CB):
        for c0 in range(128):
            esel[2 * j + c0 // HD, j, c0] = 1.0
    ident = np.eye(128, dtype=np.float32)

    in_maps = []
    for core in range(8):
        b, jh = divmod(core, 2)
        sl = slice(jh * NL, (jh + 1) * NL)
        ot = slice((1 - jh) * NL, (1 - jh) * NL + NL)
        xT = x[b].T  # [f, n]
        xt_c = np.ascontiguousarray(np.concatenate([xT[:, sl], xT[:, ot]], axis=1))
        fl_c = np.ascontiguousarray(
            factor_l[b, :, sl, :].transpose(1, 0, 2)).astype(_bf16)
        fr_c = np.ascontiguousarray(
            factor_r[b, :, sl, :].transpose(1, 0, 2)).astype(_bf16)
        in_maps.append({
            "xt": xt_c, "wvtb": wvtb, "wot": wot,
            "fl": fl_c, "fr": fr_c, "esel": esel, "ident": ident,
        })
    return in_maps


def assemble(results):
    y = np.empty((B, N, D), dtype=np.float32)
    for core in range(8):
        b, jh = divmod(core, 2)
        y[b, jh * NL:(jh + 1) * NL, :] = results[core]["y"]
    return y


def kernel(x, factor_l, factor_r, Wv, Wo, _trace=False, **trace_kw):
    nc = get_nc()
    in_maps = make_in_maps(x, factor_l, factor_r, Wv, Wo)
    res = run_bass_kernel_spmd(nc, in_maps, core_ids=list(range(8)),
                               trace=_trace, **trace_kw)
    out = assemble(res.results)
    if _trace:
        return out, res
    return out


if __name__ == "__main__":
    # quick CoreSim check of core 0 and core 5
    from concourse.bass_interp import CoreSim
    import reference as REF

    inputs = {k: np.asarray(v) for k, v in REF.setup_inputs().items()}
    nc = get_nc()
    in_maps = make_in_maps(**inputs)

    # numpy reference (closed form validated against jax reference separately)
    x, fl, fr, Wv, Wo = (inputs["x"], inputs["factor_l"], inputs["factor_r"],
                         inputs["Wv"], inputs["Wo"])
    val = x @ Wv.T
    d = (fl * fr).sum(-1)
    e = np.exp(d)
    Z = e + (N - 1)
    S = val.reshape(B, N, H, HD).sum(1)
    a = (e - 1) / Z
    bb = 1 / Z
    v = val.reshape(B, N, H, HD).transpose(0, 2, 1, 3)
    out = a[..., None] * v + bb[..., None] * S[:, :, None, :]
    out = out.transpose(0, 2, 1, 3).reshape(B, N, D)
    want_full = out @ Wo.T

    for core in [0, 5]:
        sim = CoreSim(nc)
        for k2, v2 in in_maps[core].items():
            sim.tensor(k2)[:] = v2
        sim.simulate()
        got = np.array(sim.tensor("y"))
        b, jh = divmod(core, 2)
        want = want_full[b, jh * NL:(jh + 1) * NL, :]
        err = np.abs(got - want).max() / np.abs(want).max()
        print(f"core {core}: sim rel err {err:.3e}")


# revision 23
# speedup vs baseline: 243.1784x; 1.0332x over previous
"""Trainium2 Bass kernel for nn_MultiHeadFactorizedRandomAttention.

Math: the reference builds scores = diag(sum_r l*r) (an [N,N] diagonal
matrix per (b,h)) and softmaxes it. A diagonal-score softmax has the
closed form

    out_i = ((e^{d_i} - 1) * v_i + sum_j v_j) / (e^{d_i} + N - 1)

so the O(N^2) attention collapses to two dense projections (x @ Wv.T,
out @ Wo.T) plus per-(head, position) scaling and a per-head column sum
of v.  Sharding: 8 cores = 4 batches x 2 sequence halves; every core
computes y[b, n_half, :] independently (no collectives).

Per-core device program (matmuls in float32r, 1 cycle/row at N>=256;
factor tensors ship bf16 since they only form the attention scores):
  valueT[c, n]   = sum_f WvT[f, c] * xT[f, n]          (c-block j, k-loop over f)
  S[c]           = sum_f WvT[f, c] * xs[f]             (xs = colsum of xT, on-chip)
  d[n, h]        = sum_r fl*fr ; e = exp(d)
  a = (e-1)/(e+N-1), b = 1/(e+N-1)   -> PE-transposed to [h, n]
  A_rep[c, n]    = E_j.T @ a_hn  (selector matmul replicates head rows)
  outT[c, n]     = valueT * A_rep + B_rep * S[c]
  y[n, c']       = sum_c outT[c, n] * WoT[c, c']
"""

import numpy as np
from ml_dtypes import bfloat16 as _bf16
from contextlib import ExitStack

import concourse.bass as bass
import concourse.mybir as mybir
from concourse import bacc, tile
from concourse.bass_utils import run_bass_kernel_spmd

DT = mybir.dt.float32
BF16 = mybir.dt.bfloat16
F32R = mybir.dt.float32r
AL = bass.mybir.AluOpType
AF = mybir.ActivationFunctionType
AX = mybir.AxisListType

B, H, N, R, D = 4, 16, 1024, 64, 1024
HD = D // H          # 64
NL = N // 2          # 512 rows per core
KB = 8               # f (contraction) blocks of 128
CB = 8               # c blocks of 128
NT = NL // 128       # 4 n-tiles of 128


def build_nc():
    nc = bacc.Bacc("TRN2", target_bir_lowering=False, debug=False)

    xt = nc.dram_tensor("xt", [D, N], F32R, kind="ExternalInput")        # x[b].T, local n first
    wvtb = nc.dram_tensor("wvtb", [CB, 128, KB, 128], F32R, kind="ExternalInput")  # [j, f0, k, c0]
    wot = nc.dram_tensor("wot", [D, D], F32R, kind="ExternalInput")      # Wo.T  [c, c']
    # factors ship as bf16: they only produce the scores d = sum_r l*r
    # (attention weights); their error contribution to y is ~1e-6 relative.
    fl = nc.dram_tensor("fl", [NL, H, R], BF16, kind="ExternalInput")    # [n, h, r]
    fr = nc.dram_tensor("fr", [NL, H, R], BF16, kind="ExternalInput")
    esel = nc.dram_tensor("esel", [H, CB, 128], F32R, kind="ExternalInput")
    ident = nc.dram_tensor("ident", [128, 128], DT, kind="ExternalInput")
    y = nc.dram_tensor("y", [NL, D], DT, kind="ExternalOutput")

    with tile.TileContext(nc) as tc, ExitStack() as ctx:
        const = ctx.enter_context(tc.tile_pool(name="const", bufs=1))
        xt_pool = ctx.enter_context(tc.tile_pool(name="xt", bufs=1))
        wvt_pool = ctx.enter_context(tc.tile_pool(name="wvt", bufs=1))
        wot_pool = ctx.enter_context(tc.tile_pool(name="wot", bufs=1))
        fct_pool = ctx.enter_context(tc.tile_pool(name="fct", bufs=2))
        small = ctx.enter_context(tc.tile_pool(name="small", bufs=2))
        tmp_pool = ctx.enter_context(tc.tile_pool(name="tmp", bufs=2))
        out_pool = ctx.enter_context(tc.tile_pool(name="outT", bufs=CB))
        ysb_pool = ctx.enter_context(tc.tile_pool(name="ysb", bufs=4))

        # ---- constants / inputs ----
        id_sb = const.tile([128, 128], DT, tag="ident")
        nc.sync.dma_start(id_sb[:], ident[:])
        esel_sb = const.tile([H, CB, 128], F32R, tag="esel")
        nc.sync.dma_start(esel_sb[:], esel[:])

        wvt_sb = [None] * CB
        def load_wvt(j):
            t = wvt_pool.tile([128, KB, 128], F32R, tag=f"wvt{j}")
            nc.sync.dma_start(t[:], wvtb[j, :, :, :])
            wvt_sb[j] = t

        wot_sb = [None] * CB
        def load_wot(j):
            t = wot_pool.tile([128, D], F32R, tag=f"wot{j}")
            nc.sync.dma_start(t[:], wot[j * 128:(j + 1) * 128, :])
            wot_sb[j] = t

        fl_sb, fr_sb = [], []
        def load_fct(t):
            a = fct_pool.tile([128, H, R], BF16, tag="fl", bufs=NT, name=f"fl{t}")
            nc.sync.dma_start(a[:], fl[t * 128:(t + 1) * 128, :, :])
            fl_sb.append(a)
            b_ = fct_pool.tile([128, H, R], BF16, tag="fr", bufs=NT, name=f"fr{t}")
            nc.sync.dma_start(b_[:], fr[t * 128:(t + 1) * 128, :, :])
            fr_sb.append(b_)

        load_wvt(0)
        load_wvt(1)
        xt_sb = []
        for k in range(KB):
            t = xt_pool.tile([128, N], F32R, tag=f"xt{k}")
            nc.sync.dma_start(t[:], xt[k * 128:(k + 1) * 128, :])
            xt_sb.append(t)
        for t_ in range(NT):
            load_fct(t_)

        # wvt0/1 BEFORE xt so kloop0's PE matmuls stream with the xt_k
        # arrivals (PE starts at ~3.5us instead of ~17us); factors right
        # after xt (transpose chain feeds the first combine); wvt2-7 pace
        # the remaining kloops (PE j-step ~3.2us > 1.5us arrival spacing);
        # wot last (its consumer tail is shortest: MM2 rounds + output).
        for j in range(2, CB):
            load_wvt(j)
            load_wot(j - 2)
        load_wot(6)
        load_wot(7)

        # ---- xs = column sums of x (over all N), in f-partition layout ----
        # (padded to 2 columns per k: fp32r matmul needs an even moving free dim)
        xs = const.tile([128, KB, 2], F32R, tag="xs")
        nc.gpsimd.memset(xs[:].bitcast(DT), 0.0)
        xs_dump = fct_pool.tile([128, N], DT, tag="xsdump", bufs=1)
        with nc.allow_low_precision(reason="f32r is 4-byte; accum is fp32"):
            for k in range(KB):
                nc.scalar.activation(xs_dump[:], xt_sb[k][:], AF.Copy,
                                     accum_out=xs[:, k, 0:1])

        # ---- factor math: d = sum_r fl*fr ; a/b coefficients ----
        a_hn = const.tile([H, NL], F32R, tag="a_hn")
        b_hn = const.tile([H, NL], F32R, tag="b_hn")
        ab_small = []   # (a_t, b_t) in [n, h] layout per n-tile
        for t in range(NT):
            prod = fct_pool.tile([128, H, R], DT, tag="prod")
            nc.vector.tensor_mul(prod[:], fl_sb[t][:], fr_sb[t][:])
            d_t = small.tile([128, H], DT, tag="d")
            nc.vector.reduce_sum(d_t[:], prod[:], axis=AX.X)
            e_t = small.tile([128, H], DT, tag="e")
            nc.scalar.activation(e_t[:], d_t[:], AF.Exp)
            den = small.tile([128, H], DT, tag="den")
            nc.vector.tensor_scalar(den[:], e_t[:], float(N - 1), None, AL.add)
            b_t = small.tile([128, H], DT, tag="bt")
            nc.vector.reciprocal(b_t[:], den[:])
            # a = (e-1)/(e+N-1) = 1 - N*b  (single fused op)
            a_t = small.tile([128, H], DT, tag="at")
            nc.vector.tensor_scalar(a_t[:], b_t[:], float(-N), 1.0, AL.mult, AL.add)
            ab_small.append((a_t, b_t))

        # ---- MM1 + combine + MM2, software-pipelined over c-blocks ----
        # PSUM (8 banks): pv 1 + S 1 + rep 2 + 4 inline y banks (i=0,1).
        # y rounds lag one c-block behind MM1 so the PE never waits on the
        # DVE combine.  i=2,3 accumulate in a deferred pass reusing slots.
        ps_v = ctx.enter_context(tc.tile_pool(name="ps_v", bufs=1, space="PSUM"))
        ps_s = ctx.enter_context(tc.tile_pool(name="ps_s", bufs=1, space="PSUM"))
        ps_rep = ctx.enter_context(tc.tile_pool(name="ps_rep", bufs=1, space="PSUM"))
        ps_y = ctx.enter_context(tc.tile_pool(name="ps_y", bufs=4, space="PSUM"))

        N_INLINE = 2
        inline_i = list(range(N_INLINE))
        defer_i = list(range(N_INLINE, NT))
        outT = []
        y_ps = {}

        def kloop(j):
            pv = ps_v.tile([128, NL], DT, tag="pv")
            ps = ps_s.tile([128, 2], DT, tag="ps")
            for k in range(KB):
                lhs = wvt_sb[j][:, k, :]
                nc.tensor.matmul(pv[:], lhs, xt_sb[k][:, 0:NL],
                                 start=(k == 0), stop=(k == KB - 1))
                nc.tensor.matmul(ps[:], lhs, xs[:, k, :],
                                 start=(k == 0), stop=(k == KB - 1))
            return pv, ps

        def transposes():
            for t in range(NT):
                a_t, b_t = ab_small[t]
                for src_, dst in ((a_t, a_hn), (b_t, b_hn)):
                    tp = ps_y.tile([H, 128], DT, tag="ypsum", name="tp")
                    nc.tensor.transpose(tp[:], src_[:], id_sb[:])
                    nc.scalar.copy(dst[:, t * 128:(t + 1) * 128], tp[:])

        def rep_mms(j):
            arep = ps_rep.tile([128, NL], DT, tag="arep")
            nc.tensor.matmul(arep[:], esel_sb[:, j, :], a_hn[:], start=True, stop=True)
            brep = ps_rep.tile([128, NL], DT, tag="brep")
            nc.tensor.matmul(brep[:], esel_sb[:, j, :], b_hn[:], start=True, stop=True)
            return arep, brep

        def combine(j, pv, ps, arep, brep):
            s_sb = small.tile([128, 1], DT, tag="ssb")
            nc.scalar.copy(s_sb[:], ps[:, 0:1])
            v_sb = tmp_pool.tile([128, NL], DT, tag="vsb")
            nc.vector.tensor_copy(v_sb[:], pv[:])
            t1 = tmp_pool.tile([128, NL], DT, tag="t1")
            nc.vector.tensor_mul(t1[:], v_sb[:], arep[:])
            o = out_pool.tile([128, NL], F32R, tag="outT")
            nc.vector.scalar_tensor_tensor(o[:], brep[:], s_sb[:], t1[:],
                                           AL.mult, AL.add)
            outT.append(o)

        def y_round(j, i_list):
            for i in i_list:
                lhs = outT[j][:, i * 128:(i + 1) * 128]
                for h in range(2):
                    if j == 0:
                        y_ps[i * 2 + h] = ps_y.tile([128, 512], DT, tag="ypsum",
                                                    name=f"y_ps{i}_{h}")
                    nc.tensor.matmul(y_ps[i * 2 + h][:], lhs,
                                     wot_sb[j][:, h * 512:(h + 1) * 512],
                                     start=(j == 0), stop=(j == CB - 1))

        def y_out(i):
            y_sb = ysb_pool.tile([128, D], DT, tag="ysb")
            nc.vector.tensor_copy(y_sb[:, 0:512], y_ps[i * 2][:])
            nc.vector.tensor_copy(y_sb[:, 512:1024], y_ps[i * 2 + 1][:])
            nc.sync.dma_start(y[i * 128:(i + 1) * 128, :], y_sb[:])

        pend = {}
        pend[0] = kloop(0)
        transposes()
        pend[0] += rep_mms(0)
        combine(0, *pend.pop(0))
        for j in range(1, CB):
            pv, ps = kloop(j)
            arep, brep = rep_mms(j)
            y_round(j - 1, inline_i)     # previous block's inline MM2 round
            combine(j, pv, ps, arep, brep)
        y_round(CB - 1, inline_i)
        for i in inline_i:
            y_out(i)
        # phase B: deferred i-tiles (all operands SBUF-resident)
        for j in range(CB):
            y_round(j, defer_i)
        for i in defer_i:
            y_out(i)

    nc.compile()
    return nc


_NC_CACHE = None


def get_nc():
    global _NC_CACHE
    if _NC_CACHE is None:
        _NC_CACHE = build_nc()
    return _NC_CACHE


def make_in_maps(x, factor_l, factor_r, Wv, Wo):
    x = np.asarray(x, dtype=np.float32)
    factor_l = np.asarray(factor_l, dtype=np.float32)
    factor_r = np.asarray(factor_r, dtype=np.float32)
    Wv = np.asarray(Wv, dtype=np.float32)
    Wo = np.asarray(Wo, dtype=np.float32)

    wvt = Wv.T  # [f, c]
    # wvtb[j, f0, k, c0] = WvT[k*128+f0, j*128+c0]
    wvtb = np.ascontiguousarray(
        wvt.reshape(KB, 128, CB, 128).transpose(2, 1, 0, 3))
    wot = np.ascontiguousarray(Wo.T)

    esel = np.zeros((H, CB, 128), dtype=np.float32)
    for j in range(CB):
        for c0 in range(128):
            esel[2 * j + c0 // HD, j, c0] = 1.0
    ident = np.eye(128, dtype=np.float32)

    in_maps = []
    for core in range(8):
        b, jh = divmod(core, 2)
        sl = slice(jh * NL, (jh + 1) * NL)
        ot = slice((1 - jh) * NL, (1 - jh) * NL + NL)
        xT = x[b].T  # [f, n]
        xt_c = np.ascontiguousarray(np.concatenate([xT[:, sl], xT[:, ot]], axis=1))
        fl_c = np.ascontiguousarray(
            factor_l[b, :, sl, :].transpose(1, 0, 2)).astype(_bf16)
        fr_c = np.ascontiguousarray(
            factor_r[b, :, sl, :].transpose(1, 0, 2)).astype(_bf16)
        in_maps.append({
            "xt": xt_c, "wvtb": wvtb, "wot": wot,
            "fl": fl_c, "fr": fr_c, "esel": esel, "ident": ident,
        })
    return in_maps


def assemble(results):
    y = np.empty((B, N, D), dtype=np.float32)
    for core in range(8):
        b, jh = divmod(core, 2)
        y[b, jh * NL:(jh + 1) * NL, :] = results[core]["y"]
    return y


def kernel(x, factor_l, factor_r, Wv, Wo, _trace=False, **trace_kw):
    nc = get_nc()
    in_maps = make_in_maps(x, factor_l, factor_r, Wv, Wo)
    res = run_bass_kernel_spmd(nc, in_maps, core_ids=list(range(8)),
                               trace=_trace, **trace_kw)
    out = assemble(res.results)
    if _trace:
        return out, res
    return out


if __name__ == "__main__":
    # quick CoreSim check of core 0 and core 5
    from concourse.bass_interp import CoreSim
    import reference as REF

    inputs = {k: np.asarray(v) for k, v in REF.setup_inputs().items()}
    nc = get_nc()
    in_maps = make_in_maps(**inputs)

    # numpy reference (closed form validated against jax reference separately)
    x, fl, fr, Wv, Wo = (inputs["x"], inputs["factor_l"], inputs["factor_r"],
                         inputs["Wv"], inputs["Wo"])
    val = x @ Wv.T
    d = (fl * fr).sum(-1)
    e = np.exp(d)
    Z = e + (N - 1)
    S = val.reshape(B, N, H, HD).sum(1)
    a = (e - 1) / Z
    bb = 1 / Z
    v = val.reshape(B, N, H, HD).transpose(0, 2, 1, 3)
    out = a[..., None] * v + bb[..., None] * S[:, :, None, :]
    out = out.transpose(0, 2, 1, 3).reshape(B, N, D)
    want_full = out @ Wo.T

    for core in [0, 5]:
        sim = CoreSim(nc)
        for k2, v2 in in_maps[core].items():
            sim.tensor(k2)[:] = v2
        sim.simulate()
        got = np.array(sim.tensor("y"))
        b, jh = divmod(core, 2)
        want = want_full[b, jh * NL:(jh + 1) * NL, :]
        err = np.abs(got - want).max() / np.abs(want).max()
        print(f"core {core}: sim rel err {err:.3e}")
